# revision 17
# baseline (speedup 1.0000x reference)
"""Gemma3 sliding-window attention layer on 8 Trainium2 NeuronCores.

Sharding: tensor-parallel over heads. Core c computes q-head c and kv-head c//2
(kv heads are duplicated across the 2 cores sharing them), then the o_proj
row-slice for its head. The 8 partial o_proj outputs are summed on the host
(the unshard step for row-sharded o_proj).

v2 restructure (vs baseline):
- RMSNorm rstd computed as Exp(-0.5*Ln(var)) so every ACT function used
  (square/copy/ln/exp) lives in ONE activation table -> no ACT_TABLE_LOADs.
- SCALING folded into rstd_q via the Exp bias; softmax exp runs unscaled.
- Softmax normalization moved past PV: o_proj consumes unnormalized PV
  output; 1/sums is transposed to a per-token column (two tiny matmuls)
  and folded into the per-partition scale of the o_proj PSUM evacuation
  (alternating ACT/DVE), so normalization costs no extra passes.
- Per-iteration emission order software-pipelines PE: attn(t) ->
  qkv proj(t+1) (covers softmax->o_proj chain) -> o_proj(t) (covers
  rmsnorm/rope chain of t+1). Chain-fed matmuls (rb, ssq, ib) are placed
  behind independent matmul bursts.
- PSUM packed into exactly 8 banks; PE warmup matmuls + split weight DMA
  shrink the cold-start gap.
"""
import os
import sys
import types
import contextlib
import ctypes
import math

import numpy as np

for _p in ("/opt/trn_rl_repo", "/root/.axon_site/_ro/trn_rl_repo"):
    if os.path.isdir(_p) and _p not in sys.path:
        sys.path.insert(0, _p)

from contextlib import ExitStack

import concourse.bass as bass
import concourse.mybir as mybir
import concourse.tile as tile
from concourse import bacc
from concourse.bass_utils import run_bass_kernel_spmd

S = 4096
HID = 2560
NH = 8
NKV = 4
HD = 256
WIN = 1024
ROPE_BASE = 10000.0
EPS = 1e-6
SCALING = HD ** -0.5
LNSCALE = math.log(SCALING)

NCORES = 8
CH = 256            # tokens per chunk/block
NCH = S // CH       # 16
KT = HID // 128     # 20 hid k-tiles
f32 = mybir.dt.float32
f32r = mybir.dt.float32r
AF = mybir.ActivationFunctionType
ALU = mybir.AluOpType

_NC = None
_last_results = None


def _install_ntff_shim():
    """antenv.axon_hooks is absent in this image; rebuild it over libaxon so
    run_bass_kernel_spmd(trace=True) can capture NTFF profiles."""
    if "antenv.axon_hooks" in sys.modules:
        return
    so_path = "/opt/axon/libaxon_pjrt.so"
    hook = None
    try:
        lib = ctypes.CDLL(so_path)
        if hasattr(lib, "axon_start_nrt_profile"):
            lib.axon_start_nrt_profile.argtypes = [
                ctypes.POINTER(ctypes.c_int64),
                ctypes.c_size_t,
            ]
            lib.axon_start_nrt_profile.restype = ctypes.c_int64
            lib.axon_stop_nrt_profile.argtypes = [ctypes.c_char_p]
            lib.axon_stop_nrt_profile.restype = ctypes.c_int64

            @contextlib.contextmanager
            def _hook(output_dir, device_ids):
                import jax

                jax.devices()
                if device_ids:
                    ids = (ctypes.c_int64 * len(device_ids))(*device_ids)
                    rc = lib.axon_start_nrt_profile(ids, len(device_ids))
                else:
                    rc = lib.axon_start_nrt_profile(None, 0)
                if rc != 0:
                    raise RuntimeError(f"axon_start_nrt_profile rc={rc}")
                try:
                    yield
                finally:
                    n = lib.axon_stop_nrt_profile(str(output_dir).encode())
                    if n < 0:
                        raise RuntimeError(f"axon_stop_nrt_profile rc={n}")

            hook = _hook
    except OSError:
        pass
    mod = types.ModuleType("antenv.axon_hooks")
    mod.get_axon_ntff_profile_hook = lambda: hook
    mod.set_axon_ntff_profile_hook = lambda h: None
    sys.modules["antenv.axon_hooks"] = mod


def _body(ctx, tc, hT, w, ow, cs, msk, nw, on, on1, kc, outp):
    nc = tc.nc

    const = ctx.enter_context(tc.tile_pool(name="const", bufs=1))
    hpool = ctx.enter_context(tc.tile_pool(name="hT", bufs=2))
    cspool = ctx.enter_context(tc.tile_pool(name="cs", bufs=2))
    qpool = ctx.enter_context(tc.tile_pool(name="qT", bufs=2))
    kvpool = ctx.enter_context(tc.tile_pool(name="kv", bufs=6))
    vpool = ctx.enter_context(tc.tile_pool(name="v", bufs=12))
    sqpool = ctx.enter_context(tc.tile_pool(name="sq", bufs=2))
    tmp = ctx.enter_context(tc.tile_pool(name="tmp", bufs=6))
    xpool = ctx.enter_context(tc.tile_pool(name="x", bufs=4))
    small = ctx.enter_context(tc.tile_pool(name="small", bufs=6))
    prp = ctx.enter_context(tc.tile_pool(name="pr", bufs=1))
    apool = ctx.enter_context(tc.tile_pool(name="at", bufs=2))
    obp = ctx.enter_context(tc.tile_pool(name="ob", bufs=3))

    # PSUM: 8 banks. One psum bank row may hold only ONE in-flight matmul
    # accumulation group at a time (start=True marks the whole 2KB row
    # pending-zero); sequential groups in one bank are fine.
    bigp = ctx.enter_context(tc.tile_pool(name="big", bufs=3, space="PSUM"))
    scp = ctx.enter_context(tc.tile_pool(name="scb", bufs=3, space="PSUM"))
    pvap = ctx.enter_context(tc.tile_pool(name="pva", bufs=1, space="PSUM"))
    pvbp = ctx.enter_context(tc.tile_pool(name="pvb", bufs=1, space="PSUM"))

    # resident constants; small ones first so PE warmup can start instantly
    ones_sb = const.tile([128, 2], f32r)
    nc.sync.dma_start(out=ones_sb, in_=on)
    ones1_sb = const.tile([1, 128], f32r)
    nc.sync.dma_start(out=ones1_sb, in_=on1)
    nw_sb = const.tile([128, 4], f32)
    nc.sync.dma_start(out=nw_sb, in_=nw)
    msk_sb = const.tile([128, 384], f32)
    nc.sync.dma_start(out=msk_sb, in_=msk)
    kc_sb = const.tile([1, 2], f32)
    nc.sync.dma_start(out=kc_sb, in_=kc)

    w_sb = const.tile([128, KT * 768], f32r)
    WCH = 5 * 768  # 5 k-tiles per DMA chunk
    nc.sync.dma_start(out=w_sb[:, 0:WCH], in_=w[:, 0:WCH])

    h_t = {}
    cs_t = {}

    def dma_block(t):
        ht = hpool.tile([128, KT * CH], f32r, tag="hTt")
        nc.sync.dma_start(out=ht, in_=hT[:, t * KT * CH:(t + 1) * KT * CH])
        ct = cspool.tile([128, 2 * CH], f32, tag="cst")
        nc.sync.dma_start(out=ct, in_=cs[:, t * 2 * CH:(t + 1) * 2 * CH])
        h_t[t] = ht
        cs_t[t] = ct

    dma_block(0)
    for c in range(1, 4):
        nc.sync.dma_start(out=w_sb[:, c * WCH:(c + 1) * WCH],
                          in_=w[:, c * WCH:(c + 1) * WCH])
    ow_sb = const.tile([128, 2 * HID], f32r)
    nc.sync.dma_start(out=ow_sb[:, 0:HID], in_=ow[:, 0:HID])
    nc.sync.dma_start(out=ow_sb[:, HID:2 * HID], in_=ow[:, HID:2 * HID])
    dma_block(1)

    # PE warmup: tiny matmuls keep the HAM clock gate open until weights land
    warm = bigp.tile([128, 512], f32, tag="big")
    for i in range(150):
        nc.tensor.matmul(warm[0:1, 0:2], ones_sb[:, 0:1], ones_sb,
                         start=(i == 0), stop=(i == 149))

    kv_tiles = {}
    v_tiles = {}
    qT = {}

    def emit_rsqrt(ssq, scale_out):
        """rstd = scale_out/2 * rsqrt(ssq/HD + EPS), all on DVE.

        Two division-based Newton sqrt iterations from seed (1+v)/2;
        exact enough (<4e-4) for var in [0.5, 2]."""
        v = small.tile([1, CH], f32, tag="nt")
        nc.vector.tensor_scalar(v, ssq, 1.0 / HD, EPS, ALU.mult, ALU.add)
        y1 = small.tile([1, CH], f32, tag="nt")
        nc.vector.tensor_scalar(y1, v, 0.5, 0.5, ALU.mult, ALU.add)
        r = small.tile([1, CH], f32, tag="nt")
        nc.vector.reciprocal(r, y1)
        tt = small.tile([1, CH], f32, tag="nt")
        nc.vector.tensor_mul(tt, r, v)
        s2 = small.tile([1, CH], f32, tag="nt")
        nc.vector.tensor_add(s2, y1, tt)      # = 2*sqrt(v) approx
        rr = small.tile([1, CH], f32, tag="nt")
        nc.vector.reciprocal(rr, s2)
        rstd = small.tile([1, CH], f32r, tag="rstd")
        nc.vector.tensor_scalar_mul(rstd, rr, scale_out)
        return rstd

    def emit_proj(tp, rbx, interleave):
        """qkv projection + rmsnorm + rope for block tp; when interleave is
        given, the previous block's o_proj is woven into the emission."""
        hTt = h_t[tp]
        cst = cs_t[tp]
        cos = cst[:, 0:CH]
        sin_ = cst[:, CH:2 * CH]

        qTt = qpool.tile([128, 2 * CH], f32r, tag="qTt")
        kvt = kvpool.tile([128, 2 * CH], f32r, tag="kvt")

        # ---- q projection (j-outer: one accumulation group at a time) ----
        q_ps = bigp.tile([128, 512], f32, tag="big")
        for j in range(2):
            for k in range(KT):
                nc.tensor.matmul(
                    q_ps[:, j * 256:(j + 1) * 256],
                    w_sb[:, k * 768 + j * 128:k * 768 + (j + 1) * 128],
                    hTt[:, k * CH:(k + 1) * CH],
                    start=(k == 0), stop=(k == KT - 1))

        at0 = at1 = inv_col = None
        if interleave is not None:
            # transpose sums -> per-token column (for the o_proj evac scale)
            inv_ps = scp.tile([128, CH], f32, tag="sc")
            nc.tensor.matmul(inv_ps[:, 0:2], interleave["sums_sb"][:, 0:128],
                             ones1_sb[0:1, 0:2], start=True, stop=True)
            nc.tensor.matmul(inv_ps[:, 2:4], interleave["sums_sb"][:, 128:256],
                             ones1_sb[0:1, 0:2], start=True, stop=True)
            inv_col = small.tile([128, 4], f32, tag="invc")
            nc.vector.reciprocal(inv_col, inv_ps[:, 0:4])
            # unnormalized attention output -> SBUF (o_proj stationary)
            at0 = apool.tile([128, CH], f32r, tag="at")
            at1 = apool.tile([128, CH], f32r, tag="at")
            nc.vector.tensor_copy(at0, interleave["pv0"])
            nc.vector.tensor_copy(at1, interleave["pv1"])

        # q chain on ACT: Square, weight-scaled evacs, Ln, Exp
        sq = sqpool.tile([128, 512], f32r, tag="sq")
        nc.scalar.activation(sq, q_ps, AF.Square)
        x0 = xpool.tile([128, CH], f32, tag="x")
        x1 = xpool.tile([128, CH], f32, tag="x")
        nc.scalar.activation(x0, q_ps[:, 0:256], AF.Copy, bias=0.0,
                             scale=nw_sb[:, 0:1])
        nc.scalar.activation(x1, q_ps[:, 256:512], AF.Copy, bias=0.0,
                             scale=nw_sb[:, 1:2])
        ssq_q = scp.tile([1, CH], f32, tag="sc")
        nc.tensor.matmul(ssq_q, ones_sb[:, 0:1], sq[:, 0:256],
                         start=True, stop=False)
        nc.tensor.matmul(ssq_q, ones_sb[:, 0:1], sq[:, 256:512],
                         start=False, stop=True)
        # rstd_q = SCALING/sqrt(var) via 2 Newton sqrt iters on DVE
        # (var is in [0.6, 1.7]; seed y1=(1+v)/2 gives <4e-4 rel err)
        rstd_q = emit_rsqrt(ssq_q, 2.0 * SCALING)
        # rope part A for q (DVE; only needs x0/x1/cos/sin)
        a = tmp.tile([128, CH], f32, tag="m")
        nc.vector.tensor_mul(a, x0, cos)
        b = tmp.tile([128, CH], f32, tag="m")
        nc.vector.tensor_mul(b, x1, sin_)
        e = tmp.tile([128, CH], f32, tag="m")
        nc.vector.tensor_sub(e, a, b)
        c2 = tmp.tile([128, CH], f32, tag="m")
        nc.vector.tensor_mul(c2, x1, cos)
        d2 = tmp.tile([128, CH], f32, tag="m")
        nc.vector.tensor_mul(d2, x0, sin_)
        f = tmp.tile([128, CH], f32, tag="m")
        nc.vector.tensor_add(f, c2, d2)

        # ---- k projection burst (covers q chain latency) ----
        k_ps = bigp.tile([128, 512], f32, tag="big")
        for j in range(2):
            for k in range(KT):
                nc.tensor.matmul(
                    k_ps[:, j * 256:(j + 1) * 256],
                    w_sb[:, k * 768 + (j + 2) * 128:k * 768 + (j + 3) * 128],
                    hTt[:, k * CH:(k + 1) * CH],
                    start=(k == 0), stop=(k == KT - 1))

        # rb_q broadcast (rstd_q ready by now) + q rope finish
        nc.tensor.matmul(rbx, ones1_sb, rstd_q, start=True, stop=True)
        nc.vector.tensor_mul(qTt[:, 0:CH], e, rbx)
        nc.vector.tensor_mul(qTt[:, CH:2 * CH], f, rbx)

        # k chain on ACT
        sqk = sqpool.tile([128, 512], f32r, tag="sq")
        nc.scalar.activation(sqk, k_ps, AF.Square)
        xk0 = xpool.tile([128, CH], f32, tag="x")
        xk1 = xpool.tile([128, CH], f32, tag="x")
        nc.scalar.activation(xk0, k_ps[:, 0:256], AF.Copy, bias=0.0,
                             scale=nw_sb[:, 2:3])
        nc.scalar.activation(xk1, k_ps[:, 256:512], AF.Copy, bias=0.0,
                             scale=nw_sb[:, 3:4])

        # ---- v projection burst (st-outer) ----
        v_ps = bigp.tile([128, 512], f32, tag="big")
        for st in range(2):
            for k in range(KT):
                nc.tensor.matmul(
                    v_ps[:, st * 256:(st + 1) * 256],
                    hTt[:, k * CH + st * 128:k * CH + st * 128 + 128],
                    w_sb[:, k * 768 + 512:(k + 1) * 768],
                    start=(k == 0), stop=(k == KT - 1))

        # rope part A for k (DVE; runs during v burst)
        ak = tmp.tile([128, CH], f32, tag="m")
        nc.vector.tensor_mul(ak, xk0, cos)
        bk = tmp.tile([128, CH], f32, tag="m")
        nc.vector.tensor_mul(bk, xk1, sin_)
        ek = tmp.tile([128, CH], f32, tag="m")
        nc.vector.tensor_sub(ek, ak, bk)
        ck = tmp.tile([128, CH], f32, tag="m")
        nc.vector.tensor_mul(ck, xk1, cos)
        dk = tmp.tile([128, CH], f32, tag="m")
        nc.vector.tensor_mul(dk, xk0, sin_)
        fk = tmp.tile([128, CH], f32, tag="m")
        nc.vector.tensor_add(fk, ck, dk)

        # vt evacs early so o_proj matmuls reusing v_ps's PSUM slot
        # are not blocked behind the k-side Newton chain on DVE
        vt0 = vpool.tile([128, HD], f32r, tag="v")
        nc.vector.tensor_copy(vt0, v_ps[:, 0:256])
        vt1 = vpool.tile([128, HD], f32r, tag="v")
        nc.vector.tensor_copy(vt1, v_ps[:, 256:512])
        v_tiles[2 * tp] = vt0
        v_tiles[2 * tp + 1] = vt1

        # ssq_k (sqk ready after v burst)
        ssq_k = scp.tile([1, CH], f32, tag="sc")
        nc.tensor.matmul(ssq_k, ones_sb[:, 0:1], sqk[:, 0:256],
                         start=True, stop=False)
        nc.tensor.matmul(ssq_k, ones_sb[:, 0:1], sqk[:, 256:512],
                         start=False, stop=True)
        rstd_k = emit_rsqrt(ssq_k, 2.0)

        # ---- o_proj of prev block interleaved with k-chain tail ----
        def emit_kv_finish():
            nc.tensor.matmul(rbx, ones1_sb, rstd_k, start=True, stop=True)
            nc.vector.tensor_mul(kvt[:, 0:CH], ek, rbx)
            nc.vector.tensor_mul(kvt[:, CH:2 * CH], fk, rbx)

        if interleave is not None:
            t0 = interleave["t0"]
            idx = 0
            for st in range(2):
                for hc in range(5):
                    if idx == 7:
                        emit_kv_finish()
                    op = bigp.tile([128, 512], f32, tag="big")
                    nc.tensor.matmul(op, at0[:, st * 128:(st + 1) * 128],
                                     ow_sb[:, hc * 512:(hc + 1) * 512],
                                     start=True, stop=False)
                    nc.tensor.matmul(op, at1[:, st * 128:(st + 1) * 128],
                                     ow_sb[:, HID + hc * 512:HID + (hc + 1) * 512],
                                     start=False, stop=True)
                    ob = obp.tile([128, 512], f32, tag="ob")
                    nc.scalar.activation(ob, op, AF.Copy, bias=0.0,
                                         scale=inv_col[:, 2 * st:2 * st + 1])
                    nc.sync.dma_start(
                        out=outp[t0 + st * 128:t0 + (st + 1) * 128,
                                 hc * 512:(hc + 1) * 512],
                        in_=ob)
                    idx += 1
        else:
            emit_kv_finish()

        kv_tiles[tp] = kvt
        qT[tp] = qTt

    def emit_attn(t):
        """attention for the 256 queries of block t; returns o_proj state."""
        qTt = qT[t]
        pva = pvap.tile([128, 512], f32, tag="pva")
        pvb = pvbp.tile([128, 512], f32, tag="pvb")
        pv0 = pva[:, 0:256]
        rbx = pva[:, 256:512]
        pv1 = pvb[:, 0:256]

        kts = list(range(max(0, 2 * t - 8), 2 * t + 2))
        n = len(kts)
        pr_all = prp.tile([128, 10 * CH], f32r, tag="pr")

        def emit_sc(i):
            kt = kts[i]
            ct, sb = kt // 2, kt % 2
            kvsrc = kv_tiles[ct]
            sct = scp.tile([128, CH], f32, tag="sc")
            for h in range(2):
                nc.tensor.matmul(
                    sct,
                    kvsrc[:, h * CH + sb * 128:h * CH + sb * 128 + 128],
                    qTt[:, h * CH:(h + 1) * CH],
                    start=(h == 0), stop=(h == 1))
            pr = pr_all[:, i * CH:(i + 1) * CH]
            nc.scalar.activation(pr, sct, AF.Exp)
            for sidx, qt in enumerate((2 * t, 2 * t + 1)):
                sl = slice(i * CH + sidx * 128, i * CH + (sidx + 1) * 128)
                if kt == qt:
                    m = msk_sb[:, 256:384]
                elif kt > qt or kt < qt - 8:
                    m = msk_sb[:, 128:256]
                elif kt == qt - 8:
                    m = msk_sb[:, 0:128]
                else:
                    m = None
                if m is not None:
                    nc.vector.tensor_mul(pr_all[:, sl], pr_all[:, sl], m)

        def emit_pv(i):
            first, last = (i == 0), (i == n - 1)
            pr = pr_all[:, i * CH:(i + 1) * CH]
            vt = v_tiles[kts[i]]
            nc.tensor.matmul(pv0, vt[:, 0:128], pr, start=first, stop=last)
            nc.tensor.matmul(pv1, vt[:, 128:256], pr, start=first, stop=last)

        emit_sc(0)
        for i in range(1, n):
            emit_sc(i)
            emit_pv(i - 1)
        emit_pv(n - 1)

        # sums: one batched accumulation group over all pr tiles
        sums = scp.tile([1, CH], f32, tag="sc")
        for i in range(n):
            nc.tensor.matmul(sums, ones_sb[:, 0:1],
                             pr_all[:, i * CH:(i + 1) * CH],
                             start=(i == 0), stop=(i == n - 1))
        sums_sb = small.tile([1, CH], f32r, tag="ssb")
        nc.vector.tensor_copy(sums_sb, sums)
        return {"sums_sb": sums_sb, "pv0": pv0, "pv1": pv1, "rbx": rbx,
                "t0": t * CH}

    # ---- main software-pipelined loop ----
    # prologue: proj(0) with a dedicated psum region for its broadcasts
    pva0 = pvap.tile([128, 512], f32, tag="pva")
    emit_proj(0, pva0[:, 256:512], None)

    actx = None
    for t in range(NCH):
        if t + 2 < NCH:
            dma_block(t + 2)
        actx = emit_attn(t)
        if t + 1 < NCH:
            emit_proj(t + 1, actx["rbx"], actx)

    # epilogue: o_proj for the last block
    inv_ps = scp.tile([128, CH], f32, tag="sc")
    nc.tensor.matmul(inv_ps[:, 0:2], actx["sums_sb"][:, 0:128],
                     ones1_sb[0:1, 0:2], start=True, stop=True)
    nc.tensor.matmul(inv_ps[:, 2:4], actx["sums_sb"][:, 128:256],
                     ones1_sb[0:1, 0:2], start=True, stop=True)
    inv_col = small.tile([128, 4], f32, tag="invc")
    nc.vector.reciprocal(inv_col, inv_ps[:, 0:4])
    at0 = apool.tile([128, CH], f32r, tag="at")
    at1 = apool.tile([128, CH], f32r, tag="at")
    nc.vector.tensor_copy(at0, actx["pv0"])
    nc.vector.tensor_copy(at1, actx["pv1"])
    t0 = actx["t0"]
    idx = 0
    for st in range(2):
        for hc in range(5):
            op = bigp.tile([128, 512], f32, tag="big")
            nc.tensor.matmul(op, at0[:, st * 128:(st + 1) * 128],
                             ow_sb[:, hc * 512:(hc + 1) * 512],
                             start=True, stop=False)
            nc.tensor.matmul(op, at1[:, st * 128:(st + 1) * 128],
                             ow_sb[:, HID + hc * 512:HID + (hc + 1) * 512],
                             start=False, stop=True)
            ob = obp.tile([128, 512], f32, tag="ob")
            nc.scalar.activation(ob, op, AF.Copy, bias=0.0,
                                 scale=inv_col[:, 2 * st:2 * st + 1])
            nc.sync.dma_start(
                out=outp[t0 + st * 128:t0 + (st + 1) * 128,
                         hc * 512:(hc + 1) * 512],
                in_=ob)
            idx += 1


def _build():
    nc = bacc.Bacc("TRN2", target_bir_lowering=False, debug=False,
                   num_devices=NCORES)
    hT = nc.dram_tensor("hT", [128, KT * S], f32r, kind="ExternalInput").ap()
    w = nc.dram_tensor("w", [128, KT * 768], f32r, kind="ExternalInput").ap()
    ow = nc.dram_tensor("ow", [128, 2 * HID], f32r, kind="ExternalInput").ap()
    cs = nc.dram_tensor("cs", [128, NCH * 2 * CH], f32, kind="ExternalInput").ap()
    msk = nc.dram_tensor("msk", [128, 384], f32, kind="ExternalInput").ap()
    nw = nc.dram_tensor("nw", [128, 4], f32, kind="ExternalInput").ap()
    on = nc.dram_tensor("on", [128, 2], f32r, kind="ExternalInput").ap()
    on1 = nc.dram_tensor("on1", [1, 128], f32r, kind="ExternalInput").ap()
    kc = nc.dram_tensor("kc", [1, 2], f32, kind="ExternalInput").ap()
    outp = nc.dram_tensor("outp", [S, HID], f32, kind="ExternalOutput").ap()
    with tile.TileContext(nc) as tc, ExitStack() as ctx:
        with nc.allow_low_precision(reason="float32r matmul pipeline"):
            _body(ctx, tc, hT, w, ow, cs, msk, nw, on, on1, kc, outp)
    nc.compile()
    return nc


def _get_nc():
    global _NC
    if _NC is None:
        _NC = _build()
    return _NC


def kernel(positions, hidden_states, qkv_w, o_w, q_norm_w, k_norm_w):
    global _last_results
    _install_ntff_shim()

    positions = np.asarray(positions)
    hidden_states = np.asarray(hidden_states, dtype=np.float32)
    qkv_w = np.asarray(qkv_w, dtype=np.float32)
    o_w = np.asarray(o_w, dtype=np.float32)
    q_norm_w = np.asarray(q_norm_w, dtype=np.float32)
    k_norm_w = np.asarray(k_norm_w, dtype=np.float32)
    assert np.array_equal(positions.astype(np.int64), np.arange(S)), \
        "kernel assumes contiguous arange positions (banded sliding window)"

    hT0 = hidden_states.T  # [HID, S]
    hT = np.ascontiguousarray(
        hT0.reshape(KT, 128, NCH, CH).transpose(1, 2, 0, 3).reshape(128, KT * S))

    inv_freq = 1.0 / (ROPE_BASE ** (np.arange(0, HD, 2, dtype=np.float32) / HD))
    freqs = positions.astype(np.float32)[:, None] * inv_freq[None, :]  # [S,128]
    cos_t = np.ascontiguousarray(np.cos(freqs).T.astype(np.float32))
    sin_t = np.ascontiguousarray(np.sin(freqs).T.astype(np.float32))
    csb = np.stack([cos_t.reshape(128, NCH, CH), sin_t.reshape(128, NCH, CH)],
                   axis=2)  # [128, NCH, 2, CH]
    cs = np.ascontiguousarray(csb.reshape(128, NCH * 2 * CH))

    kl = np.arange(128)[:, None]
    ql = np.arange(128)[None, :]
    edge = (kl > ql).astype(np.float32)
    diag = (kl <= ql).astype(np.float32)
    zero = np.zeros((128, 128), np.float32)
    msk = np.concatenate([edge, zero, diag], axis=1)  # [128, 384]

    nwq = 1.0 + q_norm_w
    nwk = 1.0 + k_norm_w
    nw = np.stack([nwq[:128], nwq[128:], nwk[:128], nwk[128:]], axis=1)
    nw = np.ascontiguousarray(nw.astype(np.float32))  # [128, 4]

    on = np.ones((128, 2), np.float32)
    on1 = np.ones((1, 128), np.float32)
    kc = np.array([[EPS, LNSCALE]], dtype=np.float32)

    in_maps = []
    for c in range(NCORES):
        g = c // 2
        wq = qkv_w[:, c * HD:(c + 1) * HD]
        wk = qkv_w[:, NH * HD + g * HD:NH * HD + (g + 1) * HD]
        wv = qkv_w[:, (NH + NKV) * HD + g * HD:(NH + NKV) * HD + (g + 1) * HD]
        wslice = np.concatenate([wq, wk, wv], axis=1).astype(np.float32)
        wslice = np.ascontiguousarray(
            wslice.reshape(KT, 128, 768).transpose(1, 0, 2).reshape(128, KT * 768))
        owslice = o_w[c * HD:(c + 1) * HD, :].astype(np.float32)
        owslice = np.ascontiguousarray(
            owslice.reshape(2, 128, HID).transpose(1, 0, 2).reshape(128, 2 * HID))
        in_maps.append({
            "hT": hT, "w": wslice, "ow": owslice, "cs": cs, "msk": msk,
            "nw": nw, "on": on, "on1": on1, "kc": kc,
        })

    nc = _get_nc()
    res = run_bass_kernel_spmd(nc, in_maps, list(range(NCORES)))
    _last_results = res

    out = res.results[0]["outp"].astype(np.float32).copy()
    for c in range(1, NCORES):
        out += res.results[c]["outp"]
    return out


# revision 18
# speedup vs baseline: 1.0245x; 1.0245x over previous
"""Gemma3 sliding-window attention layer on 8 Trainium2 NeuronCores.

Sharding: tensor-parallel over heads. Core c computes q-head c and kv-head c//2
(kv heads are duplicated across the 2 cores sharing them), then the o_proj
row-slice for its head. The 8 partial o_proj outputs are summed on the host
(the unshard step for row-sharded o_proj).

v2 restructure (vs baseline):
- RMSNorm rstd computed as Exp(-0.5*Ln(var)) so every ACT function used
  (square/copy/ln/exp) lives in ONE activation table -> no ACT_TABLE_LOADs.
- SCALING folded into rstd_q via the Exp bias; softmax exp runs unscaled.
- Softmax normalization moved past PV: o_proj consumes unnormalized PV
  output; 1/sums is transposed to a per-token column (two tiny matmuls)
  and folded into the per-partition scale of the o_proj PSUM evacuation
  (alternating ACT/DVE), so normalization costs no extra passes.
- Per-iteration emission order software-pipelines PE: attn(t) ->
  qkv proj(t+1) (covers softmax->o_proj chain) -> o_proj(t) (covers
  rmsnorm/rope chain of t+1). Chain-fed matmuls (rb, ssq, ib) are placed
  behind independent matmul bursts.
- PSUM packed into exactly 8 banks; PE warmup matmuls + split weight DMA
  shrink the cold-start gap.
"""
import os
import sys
import types
import contextlib
import ctypes
import math

import numpy as np

for _p in ("/opt/trn_rl_repo", "/root/.axon_site/_ro/trn_rl_repo"):
    if os.path.isdir(_p) and _p not in sys.path:
        sys.path.insert(0, _p)

from contextlib import ExitStack

import concourse.bass as bass
import concourse.mybir as mybir
import concourse.tile as tile
from concourse import bacc
from concourse.bass_utils import run_bass_kernel_spmd

S = 4096
HID = 2560
NH = 8
NKV = 4
HD = 256
WIN = 1024
ROPE_BASE = 10000.0
EPS = 1e-6
SCALING = HD ** -0.5
LNSCALE = math.log(SCALING)

NCORES = 8
CH = 256            # tokens per chunk/block
NCH = S // CH       # 16
KT = HID // 128     # 20 hid k-tiles
f32 = mybir.dt.float32
f32r = mybir.dt.float32r
AF = mybir.ActivationFunctionType
ALU = mybir.AluOpType

_NC = None
_last_results = None


def _install_ntff_shim():
    """antenv.axon_hooks is absent in this image; rebuild it over libaxon so
    run_bass_kernel_spmd(trace=True) can capture NTFF profiles."""
    if "antenv.axon_hooks" in sys.modules:
        return
    so_path = "/opt/axon/libaxon_pjrt.so"
    hook = None
    try:
        lib = ctypes.CDLL(so_path)
        if hasattr(lib, "axon_start_nrt_profile"):
            lib.axon_start_nrt_profile.argtypes = [
                ctypes.POINTER(ctypes.c_int64),
                ctypes.c_size_t,
            ]
            lib.axon_start_nrt_profile.restype = ctypes.c_int64
            lib.axon_stop_nrt_profile.argtypes = [ctypes.c_char_p]
            lib.axon_stop_nrt_profile.restype = ctypes.c_int64

            @contextlib.contextmanager
            def _hook(output_dir, device_ids):
                import jax

                jax.devices()
                if device_ids:
                    ids = (ctypes.c_int64 * len(device_ids))(*device_ids)
                    rc = lib.axon_start_nrt_profile(ids, len(device_ids))
                else:
                    rc = lib.axon_start_nrt_profile(None, 0)
                if rc != 0:
                    raise RuntimeError(f"axon_start_nrt_profile rc={rc}")
                try:
                    yield
                finally:
                    n = lib.axon_stop_nrt_profile(str(output_dir).encode())
                    if n < 0:
                        raise RuntimeError(f"axon_stop_nrt_profile rc={n}")

            hook = _hook
    except OSError:
        pass
    mod = types.ModuleType("antenv.axon_hooks")
    mod.get_axon_ntff_profile_hook = lambda: hook
    mod.set_axon_ntff_profile_hook = lambda h: None
    sys.modules["antenv.axon_hooks"] = mod


def _body(ctx, tc, hT, w, ow, cs, msk, nw, on, on1, kc, outp):
    nc = tc.nc

    const = ctx.enter_context(tc.tile_pool(name="const", bufs=1))
    hpool = ctx.enter_context(tc.tile_pool(name="hT", bufs=2))
    cspool = ctx.enter_context(tc.tile_pool(name="cs", bufs=2))
    qpool = ctx.enter_context(tc.tile_pool(name="qT", bufs=2))
    kvpool = ctx.enter_context(tc.tile_pool(name="kv", bufs=6))
    vpool = ctx.enter_context(tc.tile_pool(name="v", bufs=12))
    sqpool = ctx.enter_context(tc.tile_pool(name="sq", bufs=2))
    tmp = ctx.enter_context(tc.tile_pool(name="tmp", bufs=6))
    xpool = ctx.enter_context(tc.tile_pool(name="x", bufs=4))
    small = ctx.enter_context(tc.tile_pool(name="small", bufs=6))
    prp = ctx.enter_context(tc.tile_pool(name="pr", bufs=1))
    apool = ctx.enter_context(tc.tile_pool(name="at", bufs=2))
    obp = ctx.enter_context(tc.tile_pool(name="ob", bufs=3))

    # PSUM: 8 banks. One psum bank row may hold only ONE in-flight matmul
    # accumulation group at a time (start=True marks the whole 2KB row
    # pending-zero); sequential groups in one bank are fine.
    bigp = ctx.enter_context(tc.tile_pool(name="big", bufs=3, space="PSUM"))
    scp = ctx.enter_context(tc.tile_pool(name="scb", bufs=3, space="PSUM"))
    pvap = ctx.enter_context(tc.tile_pool(name="pva", bufs=1, space="PSUM"))
    pvbp = ctx.enter_context(tc.tile_pool(name="pvb", bufs=1, space="PSUM"))

    # resident constants; small ones first so PE warmup can start instantly
    ones_sb = const.tile([128, 2], f32r)
    nc.sync.dma_start(out=ones_sb, in_=on)
    ones1_sb = const.tile([1, 128], f32r)
    nc.sync.dma_start(out=ones1_sb, in_=on1)
    nw_sb = const.tile([128, 4], f32)
    nc.sync.dma_start(out=nw_sb, in_=nw)
    msk_sb = const.tile([128, 384], f32)
    nc.sync.dma_start(out=msk_sb, in_=msk)
    kc_sb = const.tile([1, 2], f32)
    nc.sync.dma_start(out=kc_sb, in_=kc)

    w_sb = const.tile([128, KT * 768], f32r)
    WCH = 5 * 768  # 5 k-tiles per DMA chunk
    nc.sync.dma_start(out=w_sb[:, 0:WCH], in_=w[:, 0:WCH])

    h_t = {}
    cs_t = {}

    def dma_block(t):
        ht = hpool.tile([128, KT * CH], f32r, tag="hTt")
        nc.sync.dma_start(out=ht, in_=hT[:, t * KT * CH:(t + 1) * KT * CH])
        ct = cspool.tile([128, 2 * CH], f32, tag="cst")
        nc.sync.dma_start(out=ct, in_=cs[:, t * 2 * CH:(t + 1) * 2 * CH])
        h_t[t] = ht
        cs_t[t] = ct

    dma_block(0)
    for c in range(1, 4):
        nc.sync.dma_start(out=w_sb[:, c * WCH:(c + 1) * WCH],
                          in_=w[:, c * WCH:(c + 1) * WCH])
    ow_sb = const.tile([128, 2 * HID], f32r)
    nc.sync.dma_start(out=ow_sb[:, 0:HID], in_=ow[:, 0:HID])
    nc.sync.dma_start(out=ow_sb[:, HID:2 * HID], in_=ow[:, HID:2 * HID])
    dma_block(1)

    # PE warmup: tiny matmuls keep the HAM clock gate open until weights land
    warm = bigp.tile([128, 512], f32, tag="big")
    for i in range(300):
        nc.tensor.matmul(warm[0:1, 0:2], ones_sb[:, 0:1], ones_sb,
                         start=(i == 0), stop=(i == 299))

    kv_tiles = {}
    v_tiles = {}
    qT = {}

    def emit_rsqrt(ssq, scale_out):
        """rstd = scale_out/2 * rsqrt(ssq/HD + EPS), all on DVE.

        Two division-based Newton sqrt iterations from seed (1+v)/2;
        exact enough (<4e-4) for var in [0.5, 2]."""
        v = small.tile([1, CH], f32, tag="nt")
        nc.vector.tensor_scalar(v, ssq, 1.0 / HD, EPS, ALU.mult, ALU.add)
        y1 = small.tile([1, CH], f32, tag="nt")
        nc.vector.tensor_scalar(y1, v, 0.5, 0.5, ALU.mult, ALU.add)
        r = small.tile([1, CH], f32, tag="nt")
        nc.vector.reciprocal(r, y1)
        tt = small.tile([1, CH], f32, tag="nt")
        nc.vector.tensor_mul(tt, r, v)
        s2 = small.tile([1, CH], f32, tag="nt")
        nc.vector.tensor_add(s2, y1, tt)      # = 2*sqrt(v) approx
        rr = small.tile([1, CH], f32, tag="nt")
        nc.vector.reciprocal(rr, s2)
        rstd = small.tile([1, CH], f32r, tag="rstd")
        nc.vector.tensor_scalar_mul(rstd, rr, scale_out)
        return rstd

    def emit_proj(tp, rbx, interleave):
        """qkv projection + rmsnorm + rope for block tp; when interleave is
        given, the previous block's o_proj is woven into the emission."""
        hTt = h_t[tp]
        cst = cs_t[tp]
        cos = cst[:, 0:CH]
        sin_ = cst[:, CH:2 * CH]

        qTt = qpool.tile([128, 2 * CH], f32r, tag="qTt")
        kvt = kvpool.tile([128, 2 * CH], f32r, tag="kvt")

        # ---- q projection (j-outer: one accumulation group at a time) ----
        q_ps = bigp.tile([128, 512], f32, tag="big")
        for j in range(2):
            for k in range(KT):
                nc.tensor.matmul(
                    q_ps[:, j * 256:(j + 1) * 256],
                    w_sb[:, k * 768 + j * 128:k * 768 + (j + 1) * 128],
                    hTt[:, k * CH:(k + 1) * CH],
                    start=(k == 0), stop=(k == KT - 1))

        at0 = at1 = inv_col = None
        if interleave is not None:
            # transpose sums -> per-token column (for the o_proj evac scale)
            inv_ps = scp.tile([128, CH], f32, tag="sc")
            nc.tensor.matmul(inv_ps[:, 0:2], interleave["sums_sb"][:, 0:128],
                             ones1_sb[0:1, 0:2], start=True, stop=True)
            nc.tensor.matmul(inv_ps[:, 2:4], interleave["sums_sb"][:, 128:256],
                             ones1_sb[0:1, 0:2], start=True, stop=True)
            inv_col = small.tile([128, 4], f32, tag="invc")
            nc.vector.reciprocal(inv_col, inv_ps[:, 0:4])
            # unnormalized attention output -> SBUF (o_proj stationary)
            at0 = apool.tile([128, CH], f32r, tag="at")
            at1 = apool.tile([128, CH], f32r, tag="at")
            nc.vector.tensor_copy(at0, interleave["pv0"])
            nc.vector.tensor_copy(at1, interleave["pv1"])

        # q chain on ACT: Square, weight-scaled evacs, Ln, Exp
        sq = sqpool.tile([128, 512], f32r, tag="sq")
        nc.scalar.activation(sq, q_ps, AF.Square)
        x0 = xpool.tile([128, CH], f32, tag="x")
        x1 = xpool.tile([128, CH], f32, tag="x")
        nc.scalar.activation(x0, q_ps[:, 0:256], AF.Copy, bias=0.0,
                             scale=nw_sb[:, 0:1])
        nc.scalar.activation(x1, q_ps[:, 256:512], AF.Copy, bias=0.0,
                             scale=nw_sb[:, 1:2])
        ssq_q = scp.tile([1, CH], f32, tag="sc")
        nc.tensor.matmul(ssq_q, ones_sb[:, 0:1], sq[:, 0:256],
                         start=True, stop=False)
        nc.tensor.matmul(ssq_q, ones_sb[:, 0:1], sq[:, 256:512],
                         start=False, stop=True)
        # rstd_q = SCALING/sqrt(var) via 2 Newton sqrt iters on DVE
        # (var is in [0.6, 1.7]; seed y1=(1+v)/2 gives <4e-4 rel err)
        rstd_q = emit_rsqrt(ssq_q, 2.0 * SCALING)
        # rope part A for q (DVE; only needs x0/x1/cos/sin)
        a = tmp.tile([128, CH], f32, tag="m")
        nc.vector.tensor_mul(a, x0, cos)
        b = tmp.tile([128, CH], f32, tag="m")
        nc.vector.tensor_mul(b, x1, sin_)
        e = tmp.tile([128, CH], f32, tag="m")
        nc.vector.tensor_sub(e, a, b)
        c2 = tmp.tile([128, CH], f32, tag="m")
        nc.vector.tensor_mul(c2, x1, cos)
        d2 = tmp.tile([128, CH], f32, tag="m")
        nc.vector.tensor_mul(d2, x0, sin_)
        f = tmp.tile([128, CH], f32, tag="m")
        nc.vector.tensor_add(f, c2, d2)

        # ---- k projection burst (covers q chain latency) ----
        k_ps = bigp.tile([128, 512], f32, tag="big")
        for j in range(2):
            for k in range(KT):
                nc.tensor.matmul(
                    k_ps[:, j * 256:(j + 1) * 256],
                    w_sb[:, k * 768 + (j + 2) * 128:k * 768 + (j + 3) * 128],
                    hTt[:, k * CH:(k + 1) * CH],
                    start=(k == 0), stop=(k == KT - 1))

        # k chain on ACT
        sqk = sqpool.tile([128, 512], f32r, tag="sq")
        nc.scalar.activation(sqk, k_ps, AF.Square)
        xk0 = xpool.tile([128, CH], f32, tag="x")
        xk1 = xpool.tile([128, CH], f32, tag="x")
        nc.scalar.activation(xk0, k_ps[:, 0:256], AF.Copy, bias=0.0,
                             scale=nw_sb[:, 2:3])
        nc.scalar.activation(xk1, k_ps[:, 256:512], AF.Copy, bias=0.0,
                             scale=nw_sb[:, 3:4])

        # ---- v projection burst (st-outer) ----
        v_ps = bigp.tile([128, 512], f32, tag="big")
        for st in range(2):
            for k in range(KT):
                nc.tensor.matmul(
                    v_ps[:, st * 256:(st + 1) * 256],
                    hTt[:, k * CH + st * 128:k * CH + st * 128 + 128],
                    w_sb[:, k * 768 + 512:(k + 1) * 768],
                    start=(k == 0), stop=(k == KT - 1))

        # rope part A for k (DVE; runs during v burst)
        ak = tmp.tile([128, CH], f32, tag="m")
        nc.vector.tensor_mul(ak, xk0, cos)
        bk = tmp.tile([128, CH], f32, tag="m")
        nc.vector.tensor_mul(bk, xk1, sin_)
        ek = tmp.tile([128, CH], f32, tag="m")
        nc.vector.tensor_sub(ek, ak, bk)
        ck = tmp.tile([128, CH], f32, tag="m")
        nc.vector.tensor_mul(ck, xk1, cos)
        dk = tmp.tile([128, CH], f32, tag="m")
        nc.vector.tensor_mul(dk, xk0, sin_)
        fk = tmp.tile([128, CH], f32, tag="m")
        nc.vector.tensor_add(fk, ck, dk)

        # vt evacs early so o_proj matmuls reusing v_ps's PSUM slot
        # are not blocked behind the k-side Newton chain on DVE
        vt0 = vpool.tile([128, HD], f32r, tag="v")
        nc.vector.tensor_copy(vt0, v_ps[:, 0:256])
        vt1 = vpool.tile([128, HD], f32r, tag="v")
        nc.vector.tensor_copy(vt1, v_ps[:, 256:512])
        v_tiles[2 * tp] = vt0
        v_tiles[2 * tp + 1] = vt1

        # rb_q broadcast + q rope finish (deferred so the DVE Newton
        # chain for rstd_q has the whole k+v burst to complete)
        nc.tensor.matmul(rbx, ones1_sb, rstd_q, start=True, stop=True)
        nc.vector.tensor_mul(qTt[:, 0:CH], e, rbx)
        nc.vector.tensor_mul(qTt[:, CH:2 * CH], f, rbx)

        # ssq_k (sqk ready after v burst)
        ssq_k = scp.tile([1, CH], f32, tag="sc")
        nc.tensor.matmul(ssq_k, ones_sb[:, 0:1], sqk[:, 0:256],
                         start=True, stop=False)
        nc.tensor.matmul(ssq_k, ones_sb[:, 0:1], sqk[:, 256:512],
                         start=False, stop=True)
        rstd_k = emit_rsqrt(ssq_k, 2.0)

        # ---- o_proj of prev block interleaved with k-chain tail ----
        def emit_kv_finish():
            nc.tensor.matmul(rbx, ones1_sb, rstd_k, start=True, stop=True)
            nc.vector.tensor_mul(kvt[:, 0:CH], ek, rbx)
            nc.vector.tensor_mul(kvt[:, CH:2 * CH], fk, rbx)

        if interleave is not None:
            t0 = interleave["t0"]
            idx = 0
            for st in range(2):
                for hc in range(5):
                    if idx == 8:
                        emit_kv_finish()
                    op = bigp.tile([128, 512], f32, tag="big")
                    nc.tensor.matmul(op, at0[:, st * 128:(st + 1) * 128],
                                     ow_sb[:, hc * 512:(hc + 1) * 512],
                                     start=True, stop=False)
                    nc.tensor.matmul(op, at1[:, st * 128:(st + 1) * 128],
                                     ow_sb[:, HID + hc * 512:HID + (hc + 1) * 512],
                                     start=False, stop=True)
                    ob = obp.tile([128, 512], f32, tag="ob")
                    nc.scalar.activation(ob, op, AF.Copy, bias=0.0,
                                         scale=inv_col[:, 2 * st:2 * st + 1])
                    nc.sync.dma_start(
                        out=outp[t0 + st * 128:t0 + (st + 1) * 128,
                                 hc * 512:(hc + 1) * 512],
                        in_=ob)
                    idx += 1
        else:
            emit_kv_finish()

        kv_tiles[tp] = kvt
        qT[tp] = qTt

    def emit_attn(t):
        """attention for the 256 queries of block t; returns o_proj state."""
        qTt = qT[t]
        pva = pvap.tile([128, 512], f32, tag="pva")
        pvb = pvbp.tile([128, 512], f32, tag="pvb")
        pv0 = pva[:, 0:256]
        rbx = pva[:, 256:512]
        pv1 = pvb[:, 0:256]

        kts = list(range(max(0, 2 * t - 8), 2 * t + 2))
        n = len(kts)
        pr_all = prp.tile([128, 10 * CH], f32r, tag="pr")

        def emit_sc(i):
            kt = kts[i]
            ct, sb = kt // 2, kt % 2
            kvsrc = kv_tiles[ct]
            sct = scp.tile([128, CH], f32, tag="sc")
            for h in range(2):
                nc.tensor.matmul(
                    sct,
                    kvsrc[:, h * CH + sb * 128:h * CH + sb * 128 + 128],
                    qTt[:, h * CH:(h + 1) * CH],
                    start=(h == 0), stop=(h == 1))
            pr = pr_all[:, i * CH:(i + 1) * CH]
            nc.scalar.activation(pr, sct, AF.Exp)
            for sidx, qt in enumerate((2 * t, 2 * t + 1)):
                sl = slice(i * CH + sidx * 128, i * CH + (sidx + 1) * 128)
                if kt == qt:
                    m = msk_sb[:, 256:384]
                elif kt > qt or kt < qt - 8:
                    m = msk_sb[:, 128:256]
                elif kt == qt - 8:
                    m = msk_sb[:, 0:128]
                else:
                    m = None
                if m is not None:
                    nc.vector.tensor_mul(pr_all[:, sl], pr_all[:, sl], m)

        def emit_pv(i):
            first, last = (i == 0), (i == n - 1)
            pr = pr_all[:, i * CH:(i + 1) * CH]
            vt = v_tiles[kts[i]]
            nc.tensor.matmul(pv0, vt[:, 0:128], pr, start=first, stop=last)
            nc.tensor.matmul(pv1, vt[:, 128:256], pr, start=first, stop=last)

        emit_sc(0)
        for i in range(1, n):
            emit_sc(i)
            emit_pv(i - 1)
        emit_pv(n - 1)

        # sums: one batched accumulation group over all pr tiles
        sums = scp.tile([1, CH], f32, tag="sc")
        for i in range(n):
            nc.tensor.matmul(sums, ones_sb[:, 0:1],
                             pr_all[:, i * CH:(i + 1) * CH],
                             start=(i == 0), stop=(i == n - 1))
        sums_sb = small.tile([1, CH], f32r, tag="ssb")
        nc.vector.tensor_copy(sums_sb, sums)
        return {"sums_sb": sums_sb, "pv0": pv0, "pv1": pv1, "rbx": rbx,
                "t0": t * CH}

    # ---- main software-pipelined loop ----
    # prologue: proj(0) with a dedicated psum region for its broadcasts
    pva0 = pvap.tile([128, 512], f32, tag="pva")
    emit_proj(0, pva0[:, 256:512], None)

    actx = None
    for t in range(NCH):
        if t + 2 < NCH:
            dma_block(t + 2)
        actx = emit_attn(t)
        if t + 1 < NCH:
            emit_proj(t + 1, actx["rbx"], actx)

    # epilogue: o_proj for the last block
    inv_ps = scp.tile([128, CH], f32, tag="sc")
    nc.tensor.matmul(inv_ps[:, 0:2], actx["sums_sb"][:, 0:128],
                     ones1_sb[0:1, 0:2], start=True, stop=True)
    nc.tensor.matmul(inv_ps[:, 2:4], actx["sums_sb"][:, 128:256],
                     ones1_sb[0:1, 0:2], start=True, stop=True)
    inv_col = small.tile([128, 4], f32, tag="invc")
    nc.vector.reciprocal(inv_col, inv_ps[:, 0:4])
    at0 = apool.tile([128, CH], f32r, tag="at")
    at1 = apool.tile([128, CH], f32r, tag="at")
    nc.vector.tensor_copy(at0, actx["pv0"])
    nc.vector.tensor_copy(at1, actx["pv1"])
    t0 = actx["t0"]
    idx = 0
    for st in range(2):
        for hc in range(5):
            op = bigp.tile([128, 512], f32, tag="big")
            nc.tensor.matmul(op, at0[:, st * 128:(st + 1) * 128],
                             ow_sb[:, hc * 512:(hc + 1) * 512],
                             start=True, stop=False)
            nc.tensor.matmul(op, at1[:, st * 128:(st + 1) * 128],
                             ow_sb[:, HID + hc * 512:HID + (hc + 1) * 512],
                             start=False, stop=True)
            ob = obp.tile([128, 512], f32, tag="ob")
            nc.scalar.activation(ob, op, AF.Copy, bias=0.0,
                                 scale=inv_col[:, 2 * st:2 * st + 1])
            nc.sync.dma_start(
                out=outp[t0 + st * 128:t0 + (st + 1) * 128,
                         hc * 512:(hc + 1) * 512],
                in_=ob)
            idx += 1


def _build():
    nc = bacc.Bacc("TRN2", target_bir_lowering=False, debug=False,
                   num_devices=NCORES)
    hT = nc.dram_tensor("hT", [128, KT * S], f32r, kind="ExternalInput").ap()
    w = nc.dram_tensor("w", [128, KT * 768], f32r, kind="ExternalInput").ap()
    ow = nc.dram_tensor("ow", [128, 2 * HID], f32r, kind="ExternalInput").ap()
    cs = nc.dram_tensor("cs", [128, NCH * 2 * CH], f32, kind="ExternalInput").ap()
    msk = nc.dram_tensor("msk", [128, 384], f32, kind="ExternalInput").ap()
    nw = nc.dram_tensor("nw", [128, 4], f32, kind="ExternalInput").ap()
    on = nc.dram_tensor("on", [128, 2], f32r, kind="ExternalInput").ap()
    on1 = nc.dram_tensor("on1", [1, 128], f32r, kind="ExternalInput").ap()
    kc = nc.dram_tensor("kc", [1, 2], f32, kind="ExternalInput").ap()
    outp = nc.dram_tensor("outp", [S, HID], f32, kind="ExternalOutput").ap()
    with tile.TileContext(nc) as tc, ExitStack() as ctx:
        with nc.allow_low_precision(reason="float32r matmul pipeline"):
            _body(ctx, tc, hT, w, ow, cs, msk, nw, on, on1, kc, outp)
    nc.compile()
    return nc


def _get_nc():
    global _NC
    if _NC is None:
        _NC = _build()
    return _NC


def kernel(positions, hidden_states, qkv_w, o_w, q_norm_w, k_norm_w):
    global _last_results
    _install_ntff_shim()

    positions = np.asarray(positions)
    hidden_states = np.asarray(hidden_states, dtype=np.float32)
    qkv_w = np.asarray(qkv_w, dtype=np.float32)
    o_w = np.asarray(o_w, dtype=np.float32)
    q_norm_w = np.asarray(q_norm_w, dtype=np.float32)
    k_norm_w = np.asarray(k_norm_w, dtype=np.float32)
    assert np.array_equal(positions.astype(np.int64), np.arange(S)), \
        "kernel assumes contiguous arange positions (banded sliding window)"

    hT0 = hidden_states.T  # [HID, S]
    hT = np.ascontiguousarray(
        hT0.reshape(KT, 128, NCH, CH).transpose(1, 2, 0, 3).reshape(128, KT * S))

    inv_freq = 1.0 / (ROPE_BASE ** (np.arange(0, HD, 2, dtype=np.float32) / HD))
    freqs = positions.astype(np.float32)[:, None] * inv_freq[None, :]  # [S,128]
    cos_t = np.ascontiguousarray(np.cos(freqs).T.astype(np.float32))
    sin_t = np.ascontiguousarray(np.sin(freqs).T.astype(np.float32))
    csb = np.stack([cos_t.reshape(128, NCH, CH), sin_t.reshape(128, NCH, CH)],
                   axis=2)  # [128, NCH, 2, CH]
    cs = np.ascontiguousarray(csb.reshape(128, NCH * 2 * CH))

    kl = np.arange(128)[:, None]
    ql = np.arange(128)[None, :]
    edge = (kl > ql).astype(np.float32)
    diag = (kl <= ql).astype(np.float32)
    zero = np.zeros((128, 128), np.float32)
    msk = np.concatenate([edge, zero, diag], axis=1)  # [128, 384]

    nwq = 1.0 + q_norm_w
    nwk = 1.0 + k_norm_w
    nw = np.stack([nwq[:128], nwq[128:], nwk[:128], nwk[128:]], axis=1)
    nw = np.ascontiguousarray(nw.astype(np.float32))  # [128, 4]

    on = np.ones((128, 2), np.float32)
    on1 = np.ones((1, 128), np.float32)
    kc = np.array([[EPS, LNSCALE]], dtype=np.float32)

    in_maps = []
    for c in range(NCORES):
        g = c // 2
        wq = qkv_w[:, c * HD:(c + 1) * HD]
        wk = qkv_w[:, NH * HD + g * HD:NH * HD + (g + 1) * HD]
        wv = qkv_w[:, (NH + NKV) * HD + g * HD:(NH + NKV) * HD + (g + 1) * HD]
        wslice = np.concatenate([wq, wk, wv], axis=1).astype(np.float32)
        wslice = np.ascontiguousarray(
            wslice.reshape(KT, 128, 768).transpose(1, 0, 2).reshape(128, KT * 768))
        owslice = o_w[c * HD:(c + 1) * HD, :].astype(np.float32)
        owslice = np.ascontiguousarray(
            owslice.reshape(2, 128, HID).transpose(1, 0, 2).reshape(128, 2 * HID))
        in_maps.append({
            "hT": hT, "w": wslice, "ow": owslice, "cs": cs, "msk": msk,
            "nw": nw, "on": on, "on1": on1, "kc": kc,
        })

    nc = _get_nc()
    res = run_bass_kernel_spmd(nc, in_maps, list(range(NCORES)))
    _last_results = res

    out = res.results[0]["outp"].astype(np.float32).copy()
    for c in range(1, NCORES):
        out += res.results[c]["outp"]
    return out
